# revision 28
# baseline (speedup 1.0000x reference)
import sys

sys.path.insert(0, "/opt/trn_rl_repo")

import numpy as np

# Problem constants (nn_Attention_34978213658826)
B, L, DM, NH, DH = 2, 2048, 1024, 16, 64
P = 128
LT = L // P            # 16 q/k tiles
MC = DM // P           # 8 m-chunks
G = 2                  # q-tiles per group for the z matmul
NG = LT // G
HPC = 4                # heads per core
NPAIR = 2              # head pairs per core
NEG = -1.0e30
SCH = 1024             # scores psum chunk width

_CACHE = {}


def _ts(i, n):
    return slice(i * n, (i + 1) * n)


def build_bass():
    import concourse.mybir as mybir
    import concourse.tile as tile
    from concourse import bacc

    f32 = mybir.dt.float32
    f32r = mybir.dt.float32r
    bf16 = mybir.dt.bfloat16
    AX = mybir.AxisListType
    AF = mybir.ActivationFunctionType

    nc = bacc.Bacc(None, target_bir_lowering=False)
    # x^T split hi/lo in bf16 (hi + lo ~= fp32-accurate contraction, 1 cyc/row)
    xh_d = nc.dram_tensor("xh", [DM, L], bf16, kind="ExternalInput")
    xl_d = nc.dram_tensor("xl", [DM, L], bf16, kind="ExternalInput")
    wq_h = nc.dram_tensor("wqh", [NPAIR, DM + 1, P], bf16, kind="ExternalInput")
    wq_l = nc.dram_tensor("wql", [NPAIR, DM + 1, P], bf16, kind="ExternalInput")
    wk_h = nc.dram_tensor("wkh", [NPAIR, DM + 1, P], bf16, kind="ExternalInput")
    wk_l = nc.dram_tensor("wkl", [NPAIR, DM + 1, P], bf16, kind="ExternalInput")
    wv_d = nc.dram_tensor("wv", [DM + 1, HPC * DH], bf16, kind="ExternalInput")
    wo_d = nc.dram_tensor("wo", [NPAIR, P, DM], f32, kind="ExternalInput")
    msk = nc.dram_tensor("mask", [P, P], bf16, kind="ExternalInput")
    idn = nc.dram_tensor("ident", [P, P], bf16, kind="ExternalInput")
    out = nc.dram_tensor("out", [L, DM], f32, kind="ExternalOutput")
    wu_d = nc.dram_tensor("wu", [1, 1], f32, kind="ExternalOutput")

    with tile.TileContext(nc) as tc:
        with (
            tc.tile_pool(name="const", bufs=1) as const,
            tc.tile_pool(name="w", bufs=1) as wp,
            tc.tile_pool(name="qk", bufs=1) as qkp,
            tc.tile_pool(name="vz", bufs=1) as vzp,
        ):
            ident = const.tile([P, P], bf16)
            nc.gpsimd.dma_start(ident, idn[:, :])
            mask = const.tile([P, P], bf16)
            nc.gpsimd.dma_start(mask, msk[:, :])
            ones = const.tile([1, 512], bf16)
            nc.vector.memset(ones, 1.0)

            # weights: [partition=m-row, pair, m-chunk, headcol]
            wqk = {}
            for nm, dram in (("qh", wq_h), ("ql", wq_l), ("kh", wk_h), ("kl", wk_l)):
                t = wp.tile([P, NPAIR, MC, P], bf16, name=f"w{nm}", tag=f"w{nm}")
                bb = wp.tile([1, NPAIR, P], bf16, name=f"w{nm}b", tag=f"w{nm}b")
                wqk[nm] = (t, bb, dram)
            wv_t = wp.tile([P, MC, HPC * DH], bf16)
            wv_b = wp.tile([1, HPC * DH], bf16)
            wo_t = wp.tile([P, NPAIR, DM], f32r)
            qTh = qkp.tile([P, NPAIR, L], bf16)
            qTl = qkp.tile([P, NPAIR, L], bf16)
            kTh = qkp.tile([P, NPAIR, L], bf16)
            kTl = qkp.tile([P, NPAIR, L], bf16)
            vv = vzp.tile([P, LT, HPC * DH], bf16)
            zst = [vzp.tile([P, NPAIR, G * P], f32r, name=f"zst{g}", tag=f"zst{g}") for g in range(NG)]

            # ---------------- fused stages: pools ----------------
            with (
                tc.tile_pool(name="s_ps", bufs=3, space="PSUM") as s_ps,
                tc.tile_pool(name="zo_ps", bufs=1, space="PSUM") as zo_ps,
                tc.tile_pool(name="prow", bufs=3) as prowp,
                tc.tile_pool(name="pt", bufs=2) as ptp,
                tc.tile_pool(name="stat", bufs=4) as statp,
                tc.tile_pool(name="osb", bufs=2) as osbp,
                tc.tile_pool(name="proj_ps", bufs=1, space="PSUM") as proj_ps,
            ):
                xtp_ctx = tc.tile_pool(name="xt", bufs=1)
                xtp = xtp_ctx.__enter__()
                xh = xtp.tile([P, MC, L], bf16)
                xl = xtp.tile([P, MC, L], bf16)
                # load order: first x chunks, then q weights, then the rest
                for m in (0, 1):
                    nc.gpsimd.dma_start(xh[:, m], xh_d[_ts(m, P), :])
                    nc.gpsimd.dma_start(xl[:, m], xl_d[_ts(m, P), :])
                for nm in ("qh", "ql"):
                    t, bb, dram = wqk[nm]
                    for _pr in range(NPAIR):
                        nc.gpsimd.dma_start(t[:, _pr], dram[_pr, :DM, :].rearrange("(c p) h -> p c h", p=P))
                        nc.gpsimd.dma_start(bb[:, _pr], dram[_pr, DM : DM + 1, :])
                for m in range(2, MC):
                    nc.gpsimd.dma_start(xh[:, m], xh_d[_ts(m, P), :])
                    nc.gpsimd.dma_start(xl[:, m], xl_d[_ts(m, P), :])
                for nm in ("kh", "kl"):
                    t, bb, dram = wqk[nm]
                    for _pr in range(NPAIR):
                        nc.gpsimd.dma_start(t[:, _pr], dram[_pr, :DM, :].rearrange("(c p) h -> p c h", p=P))
                        nc.gpsimd.dma_start(bb[:, _pr], dram[_pr, DM : DM + 1, :])
                nc.gpsimd.dma_start(wv_t, wv_d[:DM, :].rearrange("(c p) h -> p c h", p=P))
                nc.gpsimd.dma_start(wv_b, wv_d[DM : DM + 1, :])
                for _pr in range(NPAIR):
                    nc.gpsimd.dma_start(wo_t[:, _pr], wo_d[_pr, :, :])

                # PE warm-up: ~5us of dummy matmuls while inputs stream in,
                # so HAM reaches K=8/8 before the projection chain starts.
                wup = statp.tile([1, 4], f32, tag="wup")
                wps = proj_ps.tile([P, 512], f32, name="wps", tag="pp")
                for w_ in range(48):
                    nc.tensor.matmul(
                        wps[:, :P], lhsT=ident, rhs=mask,
                        start=(w_ == 0), stop=(w_ == 47),
                    )
                nc.vector.reduce_max(wup[:1, :1], wps[:1, :P], axis=AX.X)
                nc.gpsimd.dma_start(wu_d[:, :], wup[:1, :1])

                NQ = L // 512

                def qk_proj(pr, th, tl, bh, bl, dest_h, dest_l, scale):
                    for n in range(NQ):
                        ps = proj_ps.tile([P, 512], f32, name="pp", tag="pp")
                        for m in range(MC):
                            for vi, (lw, rx) in enumerate((
                                (th[:, pr, m, :], xh),
                                (tl[:, pr, m, :], xh),
                                (th[:, pr, m, :], xl),
                            )):
                                nc.tensor.matmul(
                                    ps, lhsT=lw, rhs=rx[:, m, _ts(n, 512)],
                                    start=(m == 0 and vi == 0), stop=False,
                                )
                        nc.tensor.matmul(
                            ps, lhsT=bh[:, pr, :], rhs=ones[:, :512],
                            start=False, stop=False,
                        )
                        nc.tensor.matmul(
                            ps, lhsT=bl[:, pr, :], rhs=ones[:, :512],
                            start=False, stop=True,
                        )
                        nc.scalar.mul(dest_h[:, pr, _ts(n, 512)], ps, scale)
                        nc.vector.scalar_tensor_tensor(
                            dest_l[:, pr, _ts(n, 512)], ps, scale,
                            dest_h[:, pr, _ts(n, 512)],
                            op0=mybir.AluOpType.mult, op1=mybir.AluOpType.subtract,
                        )

                def v_proj(lt0=0, lt1=LT):
                    for lt in range(lt0, lt1):
                        ps = s_ps.tile([P, HPC * DH], f32, name="vps", tag="s")
                        for m in range(MC):
                            nc.tensor.matmul(
                                ps, lhsT=xh[:, m, _ts(lt, P)], rhs=wv_t[:, m, :],
                                start=(m == 0), stop=False,
                            )
                        nc.tensor.matmul(
                            ps, lhsT=ones[:, :P], rhs=wv_b,
                            start=False, stop=True,
                        )
                        nc.scalar.copy(vv[:, lt, :], ps)

                ptgs = {}

                def emit_S_qtile(pr, g, s):
                    if s == 0:
                        ptgs[(pr, g)] = [ptp.tile([P, LT, G, P], bf16, name=f"ptg{h2}", tag=f"ptg{h2}") for h2 in range(2)]
                    ptg = ptgs[(pr, g)]
                    if True:
                        i = g * G + s
                        klen = (i + 1) * P
                        nch = (klen + SCH - 1) // SCH
                        sps2 = [[], []]
                        # interleave the two heads' chunk matmuls (K=64
                        # row-tiled pairs run concurrently on the PE)
                        for c in range(nch):
                            cw = min(SCH, klen - c * SCH)
                            dlo = klen - P - c * SCH  # diag block offset
                            has_diag = 0 <= dlo < cw
                            sp2 = [s_ps.tile([P, SCH], f32, name=f"sp{h2}", tag="s") for h2 in range(2)]
                            for w0 in range(0, cw, 512):
                                ww = min(512, cw - w0)
                                last_piece = w0 + 512 >= cw
                                for vi, (lq, lk) in enumerate(
                                    ((qTh, kTh), (qTl, kTh), (qTh, kTl))
                                ):
                                    for h2 in range(2):
                                        nc.tensor.matmul(
                                            sp2[h2][:, w0 : w0 + ww],
                                            lhsT=lq[_ts(h2, DH), pr, _ts(i, P)],
                                            rhs=lk[_ts(h2, DH), pr, c * SCH + w0 : c * SCH + w0 + ww],
                                            start=(vi == 0),
                                            stop=(vi == 2 and not (has_diag and last_piece)),
                                        )
                            if has_diag:
                                for h2 in range(2):
                                    nc.tensor.matmul(
                                        sp2[h2][:, dlo : dlo + P],
                                        lhsT=ident,
                                        rhs=mask,
                                        start=False,
                                        stop=True,
                                    )
                            for h2 in range(2):
                                sps2[h2].append((sp2[h2], cw))
                        for h2 in range(2):
                            # two-level softmax: exp each chunk against its
                            # LOCAL max (frees psum fast), then fold the
                            # global rescale exp(m_c - m) and 1/sum into the
                            # per-chunk normalization scalar.
                            sps = sps2[h2]
                            prow = prowp.tile([P, L], bf16)
                            negmc = statp.tile([P, 4], f32, tag="negmc")
                            sums = statp.tile([P, 4], f32, tag="sums")
                            for c, (sp, cw) in enumerate(sps):
                                nc.vector.reduce_max(
                                    negmc[:, c : c + 1], sp[:, :cw], axis=AX.X, negate=True
                                )
                                nc.scalar.activation(
                                    prow[:, c * SCH : c * SCH + cw],
                                    sp[:, :cw],
                                    AF.Exp,
                                    bias=negmc[:, c : c + 1],
                                    accum_out=sums[:, c : c + 1],
                                )
                            sinv = statp.tile([P, 1], f32, tag="sinv")
                            if nch > 1:
                                negmg = statp.tile([P, 1], f32, tag="negmg")
                                nc.vector.tensor_reduce(
                                    negmg, negmc[:, :nch], axis=AX.X, op=mybir.AluOpType.min
                                )
                                rsc = statp.tile([P, 4], f32, tag="rsc")
                                nc.scalar.activation(
                                    rsc[:, :nch], negmc[:, :nch], AF.Exp,
                                    bias=negmg, scale=-1.0,
                                )
                                ssc = statp.tile([P, 4], f32, tag="ssc")
                                nc.vector.tensor_mul(ssc[:, :nch], sums[:, :nch], rsc[:, :nch])
                                stot = statp.tile([P, 1], f32, tag="stot")
                                nc.vector.reduce_sum(stot, ssc[:, :nch], axis=AX.X)
                                nc.vector.reciprocal(sinv, stot)
                                wsc = statp.tile([P, 4], f32, tag="wsc")
                                nc.vector.tensor_scalar_mul(wsc[:, :nch], rsc[:, :nch], sinv)
                                for c, (sp, cw) in enumerate(sps):
                                    nc.vector.tensor_scalar_mul(
                                        prow[:, c * SCH : c * SCH + cw],
                                        prow[:, c * SCH : c * SCH + cw],
                                        wsc[:, c : c + 1],
                                    )
                            else:
                                nc.vector.reciprocal(sinv, sums[:, :1])
                                nc.vector.tensor_scalar_mul(
                                    prow[:, :klen], prow[:, :klen], sinv
                                )
                            nc.sync.dma_start_transpose(
                                ptg[h2][:, : i + 1, s, :], prow[:, :klen]
                            )

                def emit_Z_h(pr, g, h2):
                    ptg = ptgs[(pr, g)]
                    if True:
                        hcol = (pr * 2 + h2) * DH
                        zps = zo_ps.tile([DH, G * P], f32, name="zps", tag="zo")
                        jmax = G * (g + 1)
                        for j in range(jmax):
                            sc = max(0, j - G * g)
                            nc.tensor.matmul(
                                zps[:, sc * P :],
                                lhsT=vv[:, j, hcol : hcol + DH],
                                rhs=ptg[h2][:, j, sc:G, :],
                                start=(j == 0),
                                stop=(j == jmax - 1),
                            )
                        nc.scalar.copy(zst[g][_ts(h2, DH), pr, :], zps)

                def emit_O_qtile(g, s):
                    if True:
                        i = g * G + s
                        osb = osbp.tile([P, DM], f32)
                        for mc2 in range(2):
                            ops = zo_ps.tile([P, 512], f32, name="ops", tag="zo")
                            for pr in range(NPAIR):
                                nc.tensor.matmul(
                                    ops,
                                    lhsT=zst[g][:, pr, _ts(s, P)],
                                    rhs=wo_t[:, pr, _ts(mc2, 512)],
                                    start=(pr == 0),
                                    stop=(pr == 1),
                                )
                            nc.scalar.copy(osb[:, _ts(mc2, 512)], ops)
                        nc.gpsimd.dma_start(out[_ts(i, P), :], osb)

                # Fine-grained interleave: after each q-tile's score
                # emission, pop deferred z / out-proj work so the PE always
                # has independent matmuls while softmax (DVE+ACT+transpose)
                # drains previous tiles.
                from collections import deque

                filler = deque()
                epoch = [0]

                def pump(n, drain=False):
                    for _ in range(n):
                        if not filler:
                            return
                        if not drain and filler[0][0] > epoch[0] - 2:
                            return
                        item = filler.popleft()[1]
                        if item[0] == "Z":
                            emit_Z_h(item[1], item[2], item[3])
                            if item[3] == 1 and item[1] == 1:
                                for s_ in range(G):
                                    filler.append((epoch[0], ("O", item[2], s_)))
                        else:
                            emit_O_qtile(item[1], item[2])

                def emit_S(pr, g):
                    for s_ in range(G):
                        emit_S_qtile(pr, g, s_)
                        pump(2)
                    for h2 in range(2):
                        filler.append((epoch[0], ("Z", pr, g, h2)))
                    epoch[0] += 1

                emit_qk = {
                    ("q",): lambda pr: qk_proj(pr, wqk["qh"][0], wqk["ql"][0], wqk["qh"][1], wqk["ql"][1], qTh, qTl, 0.125),
                    ("k",): lambda pr: qk_proj(pr, wqk["kh"][0], wqk["kl"][0], wqk["kh"][1], wqk["kl"][1], kTh, kTl, 1.0),
                }
                emit_qk[("q",)](0)
                emit_qk[("k",)](0)
                emit_S(0, 0)
                v_proj(0, 10)
                emit_S(0, 1)
                emit_S(0, 2)
                emit_qk[("q",)](1)
                emit_S(0, 3)
                emit_S(0, 4)
                emit_qk[("k",)](1)
                xtp_ctx.__exit__(None, None, None)
                for g in range(5, NG):
                    # feed late V tiles as PE filler in the otherwise
                    # starved pr=0 tail (needed by z(0, g) two groups on)
                    v_proj(2 * g, 2 * g + 2)
                    emit_S(0, g)
                for g in range(NG):
                    emit_S(1, g)
                while filler:
                    pump(1, drain=True)

    nc.finalize()
    return nc


def _split_bf16(a):
    import ml_dtypes

    hi = a.astype(ml_dtypes.bfloat16)
    lo = (a - hi.astype(np.float32)).astype(ml_dtypes.bfloat16)
    return hi, lo


def make_in_maps(normal_pre_resid, W_Q, W_K, W_V, W_O, b_Q, b_K, b_V, b_O):
    import ml_dtypes

    x = np.asarray(normal_pre_resid, np.float32)
    W_Q = np.asarray(W_Q, np.float32)
    W_K = np.asarray(W_K, np.float32)
    W_V = np.asarray(W_V, np.float32)
    W_O = np.asarray(W_O, np.float32)
    b_Q = np.asarray(b_Q, np.float32)
    b_K = np.asarray(b_K, np.float32)
    b_V = np.asarray(b_V, np.float32)

    mask = np.triu(np.full((P, P), NEG, np.float32), k=1).astype(ml_dtypes.bfloat16)
    ident = np.eye(P, dtype=np.float32).astype(ml_dtypes.bfloat16)
    in_maps = []
    for c in range(8):
        b, hg = divmod(c, 4)
        heads = [4 * hg + j for j in range(HPC)]
        xT = np.ascontiguousarray(x[b].T)  # [DM, L]
        xh, xl = _split_bf16(xT)

        def pack_qk(W, bias):
            prs = []
            for p_ in range(NPAIR):
                h0, h1 = heads[2 * p_], heads[2 * p_ + 1]
                wcat = np.concatenate([W[h0], W[h1]], axis=1)  # [DM, 128]
                bcat = np.concatenate([bias[h0], bias[h1]])[None, :]
                prs.append(np.concatenate([wcat, bcat], axis=0))  # [DM+1, 128]
            return _split_bf16(np.ascontiguousarray(np.stack(prs)))

        wqh, wql = pack_qk(W_Q, b_Q)
        wkh, wkl = pack_qk(W_K, b_K)
        wv_cat = np.concatenate([W_V[h] for h in heads], axis=1)
        bv_cat = np.concatenate([b_V[h] for h in heads])[None, :]
        wv_full = np.concatenate([wv_cat, bv_cat], axis=0).astype(ml_dtypes.bfloat16)
        wo_prs = np.ascontiguousarray(
            np.stack(
                [
                    np.concatenate(
                        [W_O[heads[2 * p_]], W_O[heads[2 * p_ + 1]]], axis=0
                    )
                    for p_ in range(NPAIR)
                ]
            )
        )  # [2, 128, DM]

        in_maps.append(
            {
                "xh": np.ascontiguousarray(xh),
                "xl": np.ascontiguousarray(xl),
                "wqh": wqh,
                "wql": wql,
                "wkh": wkh,
                "wkl": wkl,
                "wv": np.ascontiguousarray(wv_full),
                "wo": wo_prs,
                "mask": mask,
                "ident": ident,
            }
        )
    return in_maps


def _enable_ldw_opt():
    import concourse.bass_utils as bu

    if getattr(bu, "_ldw_opt_patched", False):
        return
    orig = bu.run_command

    def patched(argv, **kw):
        return orig(argv, **kw)

    bu.run_command = patched
    bu._ldw_opt_patched = True


def run_device(in_maps, **kwargs):
    from concourse.bass_utils import run_bass_kernel_spmd

    _enable_ldw_opt()
    if "nc" not in _CACHE:
        _CACHE["nc"] = build_bass()
    return run_bass_kernel_spmd(_CACHE["nc"], in_maps, core_ids=list(range(8)), **kwargs)


def kernel(normal_pre_resid, W_Q, W_K, W_V, W_O, b_Q, b_K, b_V, b_O, **extra):
    b_O = np.asarray(b_O, np.float32)
    in_maps = make_in_maps(
        normal_pre_resid, W_Q, W_K, W_V, W_O, b_Q, b_K, b_V, b_O
    )
    res = run_device(in_maps)
    outs = [r["out"] for r in res.results]
    full = np.zeros((B, L, DM), np.float32)
    for c in range(8):
        full[c // 4] += outs[c]
    full += b_O[None, None, :]
    return full
